# revision 46
# baseline (speedup 1.0000x reference)
"""BiModalAttention Trainium2 kernel (v3).

Full-input contract: kernel(mode1, mode2) -> [S, B, 2D] float32.
mode1/mode2: [S=1024, B=32, D=1024] float32.

Reference computation per batch b (m1 = mode1[:, b, :], m2 = mode2[:, b, :]):
    C1 = m1 @ m2.T                  # [S, S]
    a1 = softmax_rows(C1) @ m2 * m1
    a2 = softmax_rows(C1.T) @ m1 * m2
    out[:, b, :] = concat([a1, a2], -1)

Sharding: batch dim across 8 NeuronCores (4 batch elements per core).

v3 design notes (changes vs v2, driven by the v2 ntff profile):
  * v2 lost ~114us to PE idle gaps and ~56us-equivalent to HAM 4/8-duty
    epochs that follow gap-containing epochs. The fix is a denser software
    pipeline: the transpose/softmax phase of batch j (phase A) is emitted
    interleaved at instruction granularity with the AV matmuls of batch j-1,
    and the scores phase of batch j+1 (phase CP) is interleaved with the
    exp(E2) build of batch j. The PE stream then has no dependency-gated
    stretches and the keeper matmuls of v2 are unnecessary.
  * DVE was ~47% busy (reductions + adds + AV evacuation); the Pool engine
    was idle. The softmax pre-shift adds (epre/epre2) moved to Pool
    (nc.gpsimd), in bf16 (stores pre-exponent values; exp(bf16(x)) only
    perturbs weights O(0.2%) multiplicatively, verified 3.5e-3 scale-rel).
  * All input DMA issues stay on the Pool queue but are emitted exactly at
    the point where their WAR wait is already satisfied (staggered
    prefetch), so they never head-of-line-block Pool compute. Out stores on
    sync. First batch's score operands load in need-order so C1(0) starts
    after ~2.5MB instead of 8MB.
  * PSUM: pc(2) + pt(4) + pav(2) = 8 banks exactly.

Phase structure per core (j = batch index, 4 per core):
  CP(j): for i: [Pool epre2(j-1,i); ACT exp e2(j-1,i)] + [C1(j) block i
         (16 fp32r matmuls, ACT evac, DVE row-max)]  -- PE: 29us dense
  A(j):  rm1b transposes; 8 sections t: [C2 strip transposes (PE), DVE
         col-max reduce, ACT exp+Z2-accum, Pool epre add, ACT exp E1T]
         + 4 AV(j-1) groups (PE) + ACT Z1 pass i=t; rm2b; 1/Z  -- PE: ~66us
  AV groups: [c0 dir1 i0-7][c1 dir1][c0 dir2][c1 dir2]; evac fused
         (psum * invZ[part]) * gate on DVE; stores on sync.
"""

import os
os.environ.setdefault("NEURON_RT_RESET_CORES", "1")
import time

import ml_dtypes
import numpy as np

import concourse.bacc as bacc
import concourse.mybir as mybir
import concourse.tile as tile
from concourse.masks import make_identity
from concourse.bass_utils import run_bass_kernel_spmd

S = 1024
D = 1024
B = 32
N_CORES = 8
BPC = B // N_CORES          # batch elements per core
P = 128                     # partitions
NK = S // P                 # contraction tiles (8)
NI = S // P                 # s tiles (8)
CW = 512                    # AV d-chunk width
NCH = D // CW               # AV chunks (2)

f32 = mybir.dt.float32
f32r = mybir.dt.float32r
bf16 = mybir.dt.bfloat16
AX = mybir.AxisListType
ALU = mybir.AluOpType
ACTF = mybir.ActivationFunctionType

E1_BUFS = 11   # e1(j+1)_t reuses e1(j)_(t-3): free after AV(j) dir1 (section 3)
E2_BUFS = 8    # e2(j+1)_i reuses e2(j)_i: AV(j) dir2 done before CP(j+2)
RHS_BUFS = 5   # rotation verified against staggered prefetch points
C1_BUFS = 8


def _emit_m1t(nc, sb, st, j, m1t):
    """Score operands use a (p k) d-mapping: d = p*NK + k, so each
    partition's HBM source is one 32KB contiguous run -> 128 descriptors
    for the whole tensor (the (k p) mapping needs 1024, and HWDGE desc-gen
    costs ~17ns/descriptor, ~17us for a strided 4MB load). Contraction
    over d is order-invariant as long as m1t and m2t share the mapping.
    m1t goes on gpsimd's SWDGE (fast desc-gen), m2t on the ACT hwdge
    queue (128 descs -> ~2us issue), in parallel."""
    t = sb.tile([P, NK, S], f32r, tag="m1t", bufs=1, name=f"m1t{j}")
    st["m1t"] = t
    nc.gpsimd.dma_start(out=t, in_=m1t[j].rearrange("(p k) s -> p k s", p=P))


def _emit_m2t(nc, sb, st, j, m2t):
    t = sb.tile([P, NK, S], f32r, tag="m2t", bufs=1, name=f"m2t{j}")
    st["m2t"] = t
    nc.scalar.dma_start(out=t, in_=m2t[j].rearrange("(p k) s -> p k s", p=P))


# r-chunk index -> (modality, c): AV rhs/gate tiles, natural [t_part, k, d]
_R_KEYS = (("r2", 0), ("r1", 0), ("r2", 1), ("r1", 1))


def _emit_r_load(nc, sb, st, j, which, m1n, m2n):
    """AV rhs/gate chunk. m1n/m2n are PRE-CAST to bf16 on the host: a
    casting DMA runs at ~5GB/s per DMA engine (~81GB/s per queue) and
    saturates gpsimd's queue; the cast-free bf16 load is ~2x faster and
    reads half the HBM bytes."""
    key = _R_KEYS[which]
    mn = m1n if key[0] == "r1" else m2n
    c = key[1]
    t = sb.tile([P, NK, CW], bf16, tag="rhs", bufs=RHS_BUFS,
                name=f"r{j}_{which}")
    nc.gpsimd.dma_start(
        out=t,
        in_=mn[j].rearrange("(k p) d -> p k d", p=P)[:, :, c * CW:(c + 1) * CW])
    st[key] = t


def _emit_c1_block(nc, sb, ps, st, j, i, n_list):
    m1t_sb = st["m1t"]
    m2t_sb = st["m2t"]
    c1 = st.setdefault("c1", {})
    if i not in c1:
        c1[i] = sb.tile([P, S], f32, tag="c1", bufs=C1_BUFS, name=f"c1_{j}_{i}")
    for n in n_list:
        pc = ps.tile([P, CW], f32, tag="pc", bufs=2, name=f"pc{j}_{i}_{n}")
        for k in range(NK):
            nc.tensor.matmul(
                pc,
                m1t_sb[:, k, i * P:(i + 1) * P],
                m2t_sb[:, k, n * CW:(n + 1) * CW],
                start=(k == 0),
                stop=(k == NK - 1),
            )
        nc.scalar.copy(out=c1[i][:, n * CW:(n + 1) * CW], in_=pc)


def _emit_rm1_reduce(nc, sb, st, j, i):
    if "rm1" not in st:
        st["rm1"] = sb.tile([P, NI], f32, tag="rm1", bufs=2, name=f"rm1_{j}")
    nc.vector.tensor_reduce(st["rm1"][:, i:i + 1], st["c1"][i], axis=AX.X,
                            op=ALU.max, negate=True)


def _emit_rmb(nc, sb, ps, ident, st, j, which):
    """Broadcast negated row-max across partitions: rmb[t, s] = rm[s]."""
    rm = st[which]
    rmb = sb.tile([P, S], f32, tag=which + "b", bufs=1, name=f"{which}b_{j}")
    for g in range(2):
        pt = ps.tile([P, CW], f32, tag="pt", bufs=4, name=f"{which}b_pt{j}_{g}")
        for q in range(4):
            i = g * 4 + q
            xb = sb.tile([P, P], f32, tag="xb", bufs=2,
                         name=f"{which}b_xb{j}_{i}")
            # on Pool so the copies don't queue behind DVE's phase work
            nc.gpsimd.tensor_copy(xb, rm[:, i:i + 1].broadcast_to([P, P]))
            nc.tensor.transpose(pt[:, q * P:(q + 1) * P], xb, ident)
        nc.scalar.copy(out=rmb[:, g * CW:(g + 1) * CW], in_=pt)
    st[which + "b"] = rmb


def _emit_trans_part1(nc, sb, ps, ident, st, j, t):
    """C2 strip t: PE transposes -> rm2 partials -> Z2 accum passes."""
    c1 = st["c1"]
    rm2p, rm2, z2p = st["rm2p"], st["rm2"], st["z2p"]
    pts = []
    for g in range(2):
        pt = ps.tile([P, CW], f32, tag="pt", bufs=4, name=f"pt{j}_{t}_{g}")
        pts.append(pt)
        for q in range(4):
            i = g * 4 + q
            nc.tensor.transpose(pt[:, q * P:(q + 1) * P],
                                c1[i][:, t * P:(t + 1) * P], ident)
        nc.vector.tensor_reduce(rm2p[:, 2 * t + g:2 * t + g + 1], pts[g],
                                axis=AX.X, op=ALU.max, negate=True)
    nc.vector.tensor_tensor(rm2[:, t:t + 1], rm2p[:, 2 * t:2 * t + 1],
                            rm2p[:, 2 * t + 1:2 * t + 2], op=ALU.min)
    for g in range(2):
        # Z2 partial straight from PSUM; exp output discarded
        scrz = sb.tile([P, CW], bf16, tag="scr", bufs=2,
                       name=f"scrz_{j}_{t}_{g}")
        nc.scalar.activation(scrz, pts[g], ACTF.Exp, bias=rm2[:, t:t + 1],
                             accum_out=z2p[:, 2 * t + g:2 * t + g + 1])
    return pts


def _emit_trans_part2(nc, sb, st, j, t, pts):
    """E1T strip: pre-shift add (DVE; reads PSUM) + exp. Emitted after two
    AV groups so their evacuations precede these adds in the DVE queue."""
    rm1b = st["rm1b"]
    e1 = st.setdefault("e1", {})
    e1[t] = sb.tile([P, S], bf16, tag="e1", bufs=E1_BUFS, name=f"e1_{j}_{t}")
    epre = sb.tile([P, S], bf16, tag="ep1", bufs=2, name=f"ep1_{j}_{t}")
    for g in range(2):
        nc.vector.tensor_add(epre[:, g * CW:(g + 1) * CW], pts[g],
                             rm1b[:, g * CW:(g + 1) * CW])
    nc.scalar.activation(e1[t], epre, ACTF.Exp)


def _emit_keeper(nc, ps, st, j, t):
    """Discarded fp32r matmuls: keep PE duty ~90% through A(0), where no AV
    work is available to interleave. The HAM needs high per-epoch activity
    or it halves the clock for ~5 epochs INTO the dense C1(1)/A(1) phases
    (observed 2x 17us half-clock windows with PE ~50% busy in A(0))."""
    kc = st["kc"]
    pk = ps.tile([P, CW], f32, tag="pav", bufs=2, name=f"kp{j}_{t}")
    for r in range(3):
        nc.tensor.matmul(pk, kc[:, 0:P], kc, start=(r == 0), stop=(r == 2))


def _emit_scr(nc, sb, st, j, i):
    """Z1[s] accumulation pass over c1 strip i (exp output discarded)."""
    scr = sb.tile([P, S], bf16, tag="scr", bufs=2, name=f"scr_{j}_{i}")
    nc.scalar.activation(scr, st["c1"][i], ACTF.Exp, bias=st["rm1"][:, i:i + 1],
                         accum_out=st["z1"][:, i:i + 1])


def _emit_e2(nc, sb, st, j, i):
    """E2T strip i = exp(C1 - rm2[t]) in [s_part, t] layout (AV stationary)."""
    e2 = st.setdefault("e2", {})
    e2[i] = sb.tile([P, S], bf16, tag="e2", bufs=E2_BUFS, name=f"e2_{j}_{i}")
    epre2 = sb.tile([P, S], bf16, tag="ep2", bufs=2, name=f"ep2_{j}_{i}")
    nc.gpsimd.tensor_add(epre2, st["c1"][i], st["rm2b"])
    nc.scalar.activation(e2[i], epre2, ACTF.Exp)


def _emit_av_group(nc, sb, ps, st, j, c, dirx, i, outp):
    if dirx == 1:
        es, rhs, gate, invz, dbase = (st["e1"], st[("r2", c)], st[("r1", c)],
                                      st["invz1"], 0)
    else:
        es, rhs, gate, invz, dbase = (st["e2"], st[("r1", c)], st[("r2", c)],
                                      st["invz2"], D)
    pav = ps.tile([P, CW], f32, tag="pav", bufs=2, name=f"pav{j}_{c}_{dirx}_{i}")
    for k in range(NK):
        nc.tensor.matmul(
            pav,
            es[k][:, i * P:(i + 1) * P],
            rhs[:, k, :],
            start=(k == 0),
            stop=(k == NK - 1),
        )
    a_sb = sb.tile([P, CW], f32, tag="ao", bufs=4, name=f"a{j}_{c}_{dirx}_{i}")
    nc.vector.scalar_tensor_tensor(a_sb, pav, invz[:, i:i + 1], gate[:, i, :],
                                   op0=ALU.mult, op1=ALU.mult)
    nc.sync.dma_start(
        out=outp[j, i * P:(i + 1) * P, dbase + c * CW:dbase + (c + 1) * CW],
        in_=a_sb)


def _av_group_list():
    return [(c, dirx, i)
            for (c, dirx) in ((0, 1), (1, 1), (0, 2), (1, 2))
            for i in range(NI)]


def _emit_A(nc, sb, ps, ident, sts, j, outp, m1n, m2n, m1t, m2t):
    """Phase A(j): transposes/softmax of batch j + AV of batch j-1,
    interleaved 1 transpose-group : 4 AV-groups per section."""
    st = sts[j]
    st["rm2p"] = sb.tile([P, 2 * NK], f32, tag="rm2p", bufs=2, name=f"rm2p_{j}")
    st["rm2"] = sb.tile([P, NK], f32, tag="rm2", bufs=2, name=f"rm2_{j}")
    st["z2p"] = sb.tile([P, 2 * NK], f32, tag="z2p", bufs=2, name=f"z2p_{j}")
    st["z2"] = sb.tile([P, NK], f32, tag="z2", bufs=2, name=f"z2_{j}")
    st["z1"] = sb.tile([P, NI], f32, tag="z1", bufs=2, name=f"z1_{j}")
    _emit_rmb(nc, sb, ps, ident, st, j, "rm1")
    if j + 1 < BPC:
        # batch j+1 score operands: WAR on C1(j) matmuls releases right
        # about now; the A(j) span (~66us) easily covers both transfers
        _emit_m1t(nc, sb, sts[j + 1], j + 1, m1t)
        _emit_m2t(nc, sb, sts[j + 1], j + 1, m2t)


    av = _av_group_list() if j >= 1 else []
    stp = sts[j - 1] if j >= 1 else None
    gi = 0
    for t in range(NK):
        pts = _emit_trans_part1(nc, sb, ps, ident, st, j, t)
        for _ in range(2):
            if gi < len(av):
                c, dirx, i = av[gi]
                _emit_av_group(nc, sb, ps, stp, j - 1, c, dirx, i, outp)
                gi += 1
        if not av:
            _emit_keeper(nc, ps, st, j, t)
        _emit_trans_part2(nc, sb, st, j, t, pts)
        _emit_scr(nc, sb, st, j, t)
        for _ in range(2):
            if gi < len(av):
                c, dirx, i = av[gi]
                _emit_av_group(nc, sb, ps, stp, j - 1, c, dirx, i, outp)
                gi += 1
        if t == 5 and j >= 1:
            # r(j)_1 / r(j)_2 slots freed by AV(j-1) c0-dir2 (section 5)
            _emit_r_load(nc, sb, st, j, 1, m1n, m2n)
            _emit_r_load(nc, sb, st, j, 2, m1n, m2n)
    _emit_rmb(nc, sb, ps, ident, st, j, "rm2")
    for t in range(NK):
        nc.vector.tensor_tensor(st["z2"][:, t:t + 1],
                                st["z2p"][:, 2 * t:2 * t + 1],
                                st["z2p"][:, 2 * t + 1:2 * t + 2], op=ALU.add)
    st["invz1"] = sb.tile([P, NI], f32, tag="invz1", bufs=2, name=f"invz1_{j}")
    st["invz2"] = sb.tile([P, NI], f32, tag="invz2", bufs=2, name=f"invz2_{j}")
    nc.vector.reciprocal(st["invz2"], st["z2"])
    nc.vector.reciprocal(st["invz1"], st["z1"])
    if j >= 1:
        # r(j)_3 slot freed by AV(j-1) c1-dir2 (section 7)
        _emit_r_load(nc, sb, st, j, 3, m1n, m2n)


def _emit_CP(nc, sb, ps, sts, j, m1t, m2t, m1n, m2n):
    """Phase CP(j): C1 scores of batch j (n-outer) + E2T build of j-1."""
    st, stp = sts[j], sts[j - 1]
    _emit_r_load(nc, sb, st, j, 0, m1n, m2n)
    for i in range(NI):
        _emit_e2(nc, sb, stp, j - 1, i)
        _emit_c1_block(nc, sb, ps, st, j, i, (0,))
    for i in range(NI):
        _emit_c1_block(nc, sb, ps, st, j, i, (1,))
        _emit_rm1_reduce(nc, sb, st, j, i)


def _emit_tail(nc, sb, ps, sts, outp):
    """E2T(last) build interleaved with AV(last) dir1; then dir2."""
    j = BPC - 1
    st = sts[j]
    dir1 = [(c, 1, i) for c in (0, 1) for i in range(NI)]
    gi = 0
    for i in range(NI):
        _emit_e2(nc, sb, st, j, i)
        for _ in range(2):
            c, dirx, ii = dir1[gi]
            _emit_av_group(nc, sb, ps, st, j, c, dirx, ii, outp)
            gi += 1
    for c in (0, 1):
        for i in range(NI):
            _emit_av_group(nc, sb, ps, st, j, c, 2, i, outp)


def _build():
    nc = bacc.Bacc("TRN2", target_bir_lowering=False, debug=False,
                   num_devices=N_CORES)
    m1n = nc.dram_tensor("m1n", [BPC, S, D], bf16, kind="ExternalInput").ap()
    m2n = nc.dram_tensor("m2n", [BPC, S, D], bf16, kind="ExternalInput").ap()
    m1t = nc.dram_tensor("m1t", [BPC, D, S], f32r, kind="ExternalInput").ap()
    m2t = nc.dram_tensor("m2t", [BPC, D, S], f32r, kind="ExternalInput").ap()
    outp = nc.dram_tensor("out", [BPC, S, 2 * D], f32, kind="ExternalOutput").ap()

    with tile.TileContext(nc) as tc:
        with tc.tile_pool(name="consts", bufs=1) as consts, \
             tc.tile_pool(name="sb", bufs=1) as sb, \
             tc.tile_pool(name="ps", bufs=1, space="PSUM") as ps:
            sts = [dict() for _ in range(BPC)]

            # head loads FIRST (before consts) so transfers start ASAP:
            # m1t on gpsimd/SWDGE, m2t on ACT hwdge, r(0) behind m1t
            _emit_m1t(nc, sb, sts[0], 0, m1t)
            _emit_m2t(nc, sb, sts[0], 0, m2t)
            for w in range(4):
                _emit_r_load(nc, sb, sts[0], 0, w, m1n, m2n)

            ident = consts.tile([P, P], f32)
            make_identity(nc, ident)
            kc = consts.tile([P, CW], f32r)
            nc.vector.memset(kc.bitcast(f32), 1.0)
            sts[0]["kc"] = kc

            # CP(0): C1(0)
            for i in range(NI):
                _emit_c1_block(nc, sb, ps, sts[0], 0, i, (0,))
            for i in range(NI):
                _emit_c1_block(nc, sb, ps, sts[0], 0, i, (1,))
                _emit_rm1_reduce(nc, sb, sts[0], 0, i)

            _emit_A(nc, sb, ps, ident, sts, 0, outp, m1n, m2n, m1t, m2t)
            for j in range(1, BPC):
                _emit_CP(nc, sb, ps, sts, j, m1t, m2t, m1n, m2n)
                _emit_A(nc, sb, ps, ident, sts, j, outp, m1n, m2n, m1t, m2t)
            _emit_tail(nc, sb, ps, sts, outp)
    nc.compile()
    return nc


_NC_CACHE = None


def _get_nc():
    global _NC_CACHE
    if _NC_CACHE is None:
        _NC_CACHE = _build()
    return _NC_CACHE


def kernel(mode1: np.ndarray, mode2: np.ndarray, _trace: bool = False,
           _result_box: dict | None = None) -> np.ndarray:
    mode1 = np.asarray(mode1, dtype=np.float32)
    mode2 = np.asarray(mode2, dtype=np.float32)

    m1n_all = np.ascontiguousarray(
        mode1.transpose(1, 0, 2)).astype(ml_dtypes.bfloat16)  # [B, S, D]
    m2n_all = np.ascontiguousarray(
        mode2.transpose(1, 0, 2)).astype(ml_dtypes.bfloat16)
    m1t_all = np.ascontiguousarray(mode1.transpose(1, 2, 0))  # [B, D, S]
    m2t_all = np.ascontiguousarray(mode2.transpose(1, 2, 0))

    nc = _get_nc()
    in_maps = []
    for c in range(N_CORES):
        lo, hi = c * BPC, (c + 1) * BPC
        in_maps.append({
            "m1n": m1n_all[lo:hi],
            "m2n": m2n_all[lo:hi],
            "m1t": m1t_all[lo:hi],
            "m2t": m2t_all[lo:hi],
        })

    r = None
    last_err = None
    for attempt in range(3):
        try:
            r = run_bass_kernel_spmd(nc, in_maps, list(range(N_CORES)),
                                     trace=_trace)
            break
        except Exception as e:  # transient NRT exec-unit errors recover on retry
            last_err = e
            time.sleep(2.0)
    if r is None:
        raise last_err
    if _result_box is not None:
        _result_box["result"] = r

    out = np.empty((S, B, 2 * D), dtype=np.float32)
    for c in range(N_CORES):
        res = r.results[c]["out"]  # [BPC, S, 2D]
        out[:, c * BPC:(c + 1) * BPC, :] = res.transpose(1, 0, 2)
    return out


# revision 47
# speedup vs baseline: 1.1617x; 1.1617x over previous
"""BiModalAttention Trainium2 kernel (v4).

Full-input contract: kernel(mode1, mode2) -> [S, B, 2D] float32.
mode1/mode2: [S=1024, B=32, D=1024] float32.

Reference computation per batch b (m1 = mode1[:, b, :], m2 = mode2[:, b, :]):
    C1 = m1 @ m2.T                  # [S, S]
    a1 = softmax_rows(C1) @ m2 * m1
    a2 = softmax_rows(C1.T) @ m1 * m2
    out[:, b, :] = concat([a1, a2], -1)

Sharding: batch dim across 8 NeuronCores (4 batch elements per core).

Design (v4 = measured-best config):
  * Software pipeline keeps the PE dense: phase A(j) interleaves batch j's
    C2-strip transposes/softmax with batch j-1's AV matmuls at instruction
    granularity; phase CP(j) interleaves batch j's fp32r score matmuls
    with batch j-1's E2T build. No keeper matmuls needed in steady state.
  * Per A-section emission order puts the first two AV evacuations ahead
    of the E1T pre-shift adds in the DVE queue, giving the pav PSUM banks
    ~1.8us of slack (was the source of ~1.3us/section PE stalls).
  * Pre-exponent tiles are bf16 (halves DVE write cost; exp(bf16(x)) only
    perturbs softmax weights ~0.2% multiplicatively; verified 3.5e-3
    scale-relative vs the 2e-2 gate).
  * E2T pre-shift adds run on the otherwise-idle Pool engine (SBUF-only:
    GPSIMD cannot access PSUM); the E1T adds read PSUM so they stay on DVE.
  * AV rhs/gate operands (m1n/m2n) are pre-cast to bf16 on the host:
    casting DMAs run ~5GB/s/engine and were saturating the input queue.
  * All input DMA on gpsimd's SWDGE queue (fast descriptor generation),
    emitted exactly where their buffer-reuse waits are already satisfied
    (staggered prefetch); out stores on sync's queue.
"""

import os
os.environ.setdefault("NEURON_RT_RESET_CORES", "1")
import time

import ml_dtypes
import numpy as np

import concourse.bacc as bacc
import concourse.mybir as mybir
import concourse.tile as tile
from concourse.masks import make_identity
from concourse.bass_utils import run_bass_kernel_spmd

S = 1024
D = 1024
B = 32
N_CORES = 8
BPC = B // N_CORES          # batch elements per core
P = 128                     # partitions
NK = S // P                 # contraction tiles (8)
NI = S // P                 # s tiles (8)
CW = 512                    # AV d-chunk width
NCH = D // CW               # AV chunks (2)

f32 = mybir.dt.float32
f32r = mybir.dt.float32r
bf16 = mybir.dt.bfloat16
AX = mybir.AxisListType
ALU = mybir.AluOpType
ACTF = mybir.ActivationFunctionType

E1_BUFS = 12   # e1(j+1)_t reuses e1(j)_(t-4): free after AV(j) dir1 (section 3)
E2_BUFS = 8    # e2(j+1)_i reuses e2(j)_i: AV(j) dir2 done before CP(j+2)
RHS_BUFS = 5   # rotation verified against staggered prefetch points
C1_BUFS = 8


def _emit_m_loads(nc, sb, st, j, m1t, m2t, head):
    """Score operands in d-major [d_part, k, s] layout (f32->f32r casting
    DMA on gpsimd's SWDGE queue — the only engine allowed to cast)."""
    m1t_sb = sb.tile([P, NK, S], f32r, tag="m1t", bufs=1, name=f"m1t{j}")
    m2t_sb = sb.tile([P, NK, S], f32r, tag="m2t", bufs=1, name=f"m2t{j}")
    st["m1t"], st["m2t"] = m1t_sb, m2t_sb
    a1 = m1t[j].rearrange("(k p) s -> p k s", p=P)
    a2 = m2t[j].rearrange("(k p) s -> p k s", p=P)
    if head:
        # batch 0 runs C1 with n outer / i inner; deliver in need-order
        nc.gpsimd.dma_start(out=m1t_sb[:, :, 0:P], in_=a1[:, :, 0:P])
        nc.gpsimd.dma_start(out=m2t_sb[:, :, 0:CW], in_=a2[:, :, 0:CW])
        nc.gpsimd.dma_start(out=m1t_sb[:, :, P:S], in_=a1[:, :, P:S])
        nc.gpsimd.dma_start(out=m2t_sb[:, :, CW:S], in_=a2[:, :, CW:S])
    else:
        nc.gpsimd.dma_start(out=m1t_sb, in_=a1)
        nc.gpsimd.dma_start(out=m2t_sb, in_=a2)


# r-chunk index -> (modality, c): AV rhs/gate tiles, natural [t_part, k, d]
_R_KEYS = (("r2", 0), ("r1", 0), ("r2", 1), ("r1", 1))


def _emit_r_load(nc, sb, st, j, which, m1n, m2n):
    """AV rhs/gate chunk; m1n/m2n are pre-cast to bf16 on the host."""
    key = _R_KEYS[which]
    mn = m1n if key[0] == "r1" else m2n
    c = key[1]
    t = sb.tile([P, NK, CW], bf16, tag="rhs", bufs=RHS_BUFS,
                name=f"r{j}_{which}")
    nc.gpsimd.dma_start(
        out=t,
        in_=mn[j].rearrange("(k p) d -> p k d", p=P)[:, :, c * CW:(c + 1) * CW])
    st[key] = t


def _emit_c1_block(nc, sb, ps, st, j, i, n_list):
    m1t_sb, m2t_sb = st["m1t"], st["m2t"]
    c1 = st.setdefault("c1", {})
    if i not in c1:
        c1[i] = sb.tile([P, S], f32, tag="c1", bufs=C1_BUFS, name=f"c1_{j}_{i}")
    for n in n_list:
        pc = ps.tile([P, CW], f32, tag="pc", bufs=2, name=f"pc{j}_{i}_{n}")
        for k in range(NK):
            nc.tensor.matmul(
                pc,
                m1t_sb[:, k, i * P:(i + 1) * P],
                m2t_sb[:, k, n * CW:(n + 1) * CW],
                start=(k == 0),
                stop=(k == NK - 1),
            )
        nc.scalar.copy(out=c1[i][:, n * CW:(n + 1) * CW], in_=pc)


def _emit_rm1_reduce(nc, sb, st, j, i):
    if "rm1" not in st:
        st["rm1"] = sb.tile([P, NI], f32, tag="rm1", bufs=2, name=f"rm1_{j}")
    nc.vector.tensor_reduce(st["rm1"][:, i:i + 1], st["c1"][i], axis=AX.X,
                            op=ALU.max, negate=True)


def _emit_rmb(nc, sb, ps, ident, st, j, which):
    """Broadcast negated row-max across partitions: rmb[t, s] = rm[s]."""
    rm = st[which]
    rmb = sb.tile([P, S], f32, tag=which + "b", bufs=1, name=f"{which}b_{j}")
    for g in range(2):
        pt = ps.tile([P, CW], f32, tag="pt", bufs=4, name=f"{which}b_pt{j}_{g}")
        for q in range(4):
            i = g * 4 + q
            xb = sb.tile([P, P], f32, tag="xb", bufs=2,
                         name=f"{which}b_xb{j}_{i}")
            nc.vector.tensor_copy(xb, rm[:, i:i + 1].broadcast_to([P, P]))
            nc.tensor.transpose(pt[:, q * P:(q + 1) * P], xb, ident)
        nc.scalar.copy(out=rmb[:, g * CW:(g + 1) * CW], in_=pt)
    st[which + "b"] = rmb


def _emit_trans_part1(nc, sb, ps, ident, st, j, t):
    """C2 strip t: PE transposes -> rm2 partials -> Z2 accum passes."""
    c1 = st["c1"]
    rm2p, rm2, z2p = st["rm2p"], st["rm2"], st["z2p"]
    pts = []
    for g in range(2):
        pt = ps.tile([P, CW], f32, tag="pt", bufs=4, name=f"pt{j}_{t}_{g}")
        pts.append(pt)
        for q in range(4):
            i = g * 4 + q
            nc.tensor.transpose(pt[:, q * P:(q + 1) * P],
                                c1[i][:, t * P:(t + 1) * P], ident)
        nc.vector.tensor_reduce(rm2p[:, 2 * t + g:2 * t + g + 1], pts[g],
                                axis=AX.X, op=ALU.max, negate=True)
    nc.vector.tensor_tensor(rm2[:, t:t + 1], rm2p[:, 2 * t:2 * t + 1],
                            rm2p[:, 2 * t + 1:2 * t + 2], op=ALU.min)
    for g in range(2):
        # Z2 partial straight from PSUM; exp output discarded
        scrz = sb.tile([P, CW], bf16, tag="scr", bufs=2,
                       name=f"scrz_{j}_{t}_{g}")
        nc.scalar.activation(scrz, pts[g], ACTF.Exp, bias=rm2[:, t:t + 1],
                             accum_out=z2p[:, 2 * t + g:2 * t + g + 1])
    return pts


def _emit_trans_part2(nc, sb, st, j, t, pts):
    """E1T strip: pre-shift add (DVE; reads PSUM) + exp. Emitted after two
    AV groups so their evacuations precede these adds in the DVE queue."""
    rm1b = st["rm1b"]
    e1 = st.setdefault("e1", {})
    e1[t] = sb.tile([P, S], bf16, tag="e1", bufs=E1_BUFS, name=f"e1_{j}_{t}")
    epre = sb.tile([P, S], bf16, tag="ep1", bufs=2, name=f"ep1_{j}_{t}")
    for g in range(2):
        nc.vector.tensor_add(epre[:, g * CW:(g + 1) * CW], pts[g],
                             rm1b[:, g * CW:(g + 1) * CW])
    nc.scalar.activation(e1[t], epre, ACTF.Exp)


def _emit_scr(nc, sb, st, j, i):
    """Z1[s] accumulation pass over c1 strip i (exp output discarded)."""
    scr = sb.tile([P, S], bf16, tag="scr", bufs=2, name=f"scr_{j}_{i}")
    nc.scalar.activation(scr, st["c1"][i], ACTF.Exp, bias=st["rm1"][:, i:i + 1],
                         accum_out=st["z1"][:, i:i + 1])


def _emit_e2(nc, sb, st, j, i):
    """E2T strip i = exp(C1 - rm2[t]) in [s_part, t] layout (AV stationary).
    The add runs on Pool (SBUF-only operands)."""
    e2 = st.setdefault("e2", {})
    e2[i] = sb.tile([P, S], bf16, tag="e2", bufs=E2_BUFS, name=f"e2_{j}_{i}")
    epre2 = sb.tile([P, S], bf16, tag="ep2", bufs=2, name=f"ep2_{j}_{i}")
    nc.gpsimd.tensor_add(epre2, st["c1"][i], st["rm2b"])
    nc.scalar.activation(e2[i], epre2, ACTF.Exp)


def _emit_av_group(nc, sb, ps, st, j, c, dirx, i, outp):
    if dirx == 1:
        es, rhs, gate, invz, dbase = (st["e1"], st[("r2", c)], st[("r1", c)],
                                      st["invz1"], 0)
    else:
        es, rhs, gate, invz, dbase = (st["e2"], st[("r1", c)], st[("r2", c)],
                                      st["invz2"], D)
    pav = ps.tile([P, CW], f32, tag="pav", bufs=2, name=f"pav{j}_{c}_{dirx}_{i}")
    for k in range(NK):
        nc.tensor.matmul(
            pav,
            es[k][:, i * P:(i + 1) * P],
            rhs[:, k, :],
            start=(k == 0),
            stop=(k == NK - 1),
        )
    a_sb = sb.tile([P, CW], f32, tag="ao", bufs=4, name=f"a{j}_{c}_{dirx}_{i}")
    nc.vector.scalar_tensor_tensor(a_sb, pav, invz[:, i:i + 1], gate[:, i, :],
                                   op0=ALU.mult, op1=ALU.mult)
    nc.sync.dma_start(
        out=outp[j, i * P:(i + 1) * P, dbase + c * CW:dbase + (c + 1) * CW],
        in_=a_sb)


def _av_group_list():
    return [(c, dirx, i)
            for (c, dirx) in ((0, 1), (1, 1), (0, 2), (1, 2))
            for i in range(NI)]


def _emit_A(nc, sb, ps, ident, sts, j, outp, m1n, m2n, m1t, m2t):
    """Phase A(j): transposes/softmax of batch j + AV of batch j-1,
    interleaved 1 transpose-group : 4 AV-groups per section."""
    st = sts[j]
    st["rm2p"] = sb.tile([P, 2 * NK], f32, tag="rm2p", bufs=2, name=f"rm2p_{j}")
    st["rm2"] = sb.tile([P, NK], f32, tag="rm2", bufs=2, name=f"rm2_{j}")
    st["z2p"] = sb.tile([P, 2 * NK], f32, tag="z2p", bufs=2, name=f"z2p_{j}")
    st["z2"] = sb.tile([P, NK], f32, tag="z2", bufs=2, name=f"z2_{j}")
    st["z1"] = sb.tile([P, NI], f32, tag="z1", bufs=2, name=f"z1_{j}")
    _emit_rmb(nc, sb, ps, ident, st, j, "rm1")
    if j + 1 < BPC:
        # m(j+1) loads: WAR on C1(j) matmuls releases right about now, so
        # this never head-of-line-blocks Pool compute behind it
        _emit_m_loads(nc, sb, sts[j + 1], j + 1, m1t, m2t, head=False)

    av = _av_group_list() if j >= 1 else []
    stp = sts[j - 1] if j >= 1 else None
    gi = 0
    for t in range(NK):
        pts = _emit_trans_part1(nc, sb, ps, ident, st, j, t)
        for _ in range(2):
            if gi < len(av):
                c, dirx, i = av[gi]
                _emit_av_group(nc, sb, ps, stp, j - 1, c, dirx, i, outp)
                gi += 1
        _emit_trans_part2(nc, sb, st, j, t, pts)
        _emit_scr(nc, sb, st, j, t)
        for _ in range(2):
            if gi < len(av):
                c, dirx, i = av[gi]
                _emit_av_group(nc, sb, ps, stp, j - 1, c, dirx, i, outp)
                gi += 1
        if t == 5 and j >= 1:
            # r(j)_1 / r(j)_2 slots freed by AV(j-1) c0-dir2 (section 5)
            _emit_r_load(nc, sb, st, j, 1, m1n, m2n)
            _emit_r_load(nc, sb, st, j, 2, m1n, m2n)
    _emit_rmb(nc, sb, ps, ident, st, j, "rm2")
    for t in range(NK):
        nc.vector.tensor_tensor(st["z2"][:, t:t + 1],
                                st["z2p"][:, 2 * t:2 * t + 1],
                                st["z2p"][:, 2 * t + 1:2 * t + 2], op=ALU.add)
    st["invz1"] = sb.tile([P, NI], f32, tag="invz1", bufs=2, name=f"invz1_{j}")
    st["invz2"] = sb.tile([P, NI], f32, tag="invz2", bufs=2, name=f"invz2_{j}")
    nc.vector.reciprocal(st["invz2"], st["z2"])
    nc.vector.reciprocal(st["invz1"], st["z1"])
    if j >= 1:
        # r(j)_3 slot freed by AV(j-1) c1-dir2 (section 7)
        _emit_r_load(nc, sb, st, j, 3, m1n, m2n)


def _emit_CP(nc, sb, ps, sts, j, m1t, m2t, m1n, m2n):
    """Phase CP(j): C1 scores of batch j + E2T build of batch j-1."""
    st, stp = sts[j], sts[j - 1]
    _emit_r_load(nc, sb, st, j, 0, m1n, m2n)
    for i in range(NI):
        _emit_e2(nc, sb, stp, j - 1, i)
        _emit_c1_block(nc, sb, ps, st, j, i, (0, 1))
        _emit_rm1_reduce(nc, sb, st, j, i)


def _emit_tail(nc, sb, ps, sts, outp):
    """E2T(last) build interleaved with AV(last) dir1; then dir2."""
    j = BPC - 1
    st = sts[j]
    dir1 = [(c, 1, i) for c in (0, 1) for i in range(NI)]
    gi = 0
    for i in range(NI):
        _emit_e2(nc, sb, st, j, i)
        for _ in range(2):
            c, dirx, ii = dir1[gi]
            _emit_av_group(nc, sb, ps, st, j, c, dirx, ii, outp)
            gi += 1
    for c in (0, 1):
        for i in range(NI):
            _emit_av_group(nc, sb, ps, st, j, c, 2, i, outp)


def _build():
    nc = bacc.Bacc("TRN2", target_bir_lowering=False, debug=False,
                   num_devices=N_CORES)
    m1n = nc.dram_tensor("m1n", [BPC, S, D], bf16, kind="ExternalInput").ap()
    m2n = nc.dram_tensor("m2n", [BPC, S, D], bf16, kind="ExternalInput").ap()
    m1t = nc.dram_tensor("m1t", [BPC, D, S], f32, kind="ExternalInput").ap()
    m2t = nc.dram_tensor("m2t", [BPC, D, S], f32, kind="ExternalInput").ap()
    outp = nc.dram_tensor("out", [BPC, S, 2 * D], f32, kind="ExternalOutput").ap()

    with tile.TileContext(nc) as tc:
        with tc.tile_pool(name="consts", bufs=1) as consts, \
             tc.tile_pool(name="sb", bufs=1) as sb, \
             tc.tile_pool(name="ps", bufs=1, space="PSUM") as ps:
            sts = [dict() for _ in range(BPC)]

            # head loads first so transfers start ASAP
            _emit_m_loads(nc, sb, sts[0], 0, m1t, m2t, head=True)
            for w in range(4):
                _emit_r_load(nc, sb, sts[0], 0, w, m1n, m2n)

            ident = consts.tile([P, P], f32)
            make_identity(nc, ident)

            # CP(0): C1(0), n outer so compute starts after ~2.5MB of DMA
            for n in (0, 1):
                for i in range(NI):
                    _emit_c1_block(nc, sb, ps, sts[0], 0, i, (n,))
                    if n == 1:
                        _emit_rm1_reduce(nc, sb, sts[0], 0, i)

            _emit_A(nc, sb, ps, ident, sts, 0, outp, m1n, m2n, m1t, m2t)
            for j in range(1, BPC):
                _emit_CP(nc, sb, ps, sts, j, m1t, m2t, m1n, m2n)
                _emit_A(nc, sb, ps, ident, sts, j, outp, m1n, m2n, m1t, m2t)
            _emit_tail(nc, sb, ps, sts, outp)
    nc.compile()
    return nc


_NC_CACHE = None


def _get_nc():
    global _NC_CACHE
    if _NC_CACHE is None:
        _NC_CACHE = _build()
    return _NC_CACHE


def kernel(mode1: np.ndarray, mode2: np.ndarray, _trace: bool = False,
           _result_box: dict | None = None) -> np.ndarray:
    mode1 = np.asarray(mode1, dtype=np.float32)
    mode2 = np.asarray(mode2, dtype=np.float32)

    m1n_all = np.ascontiguousarray(
        mode1.transpose(1, 0, 2)).astype(ml_dtypes.bfloat16)  # [B, S, D]
    m2n_all = np.ascontiguousarray(
        mode2.transpose(1, 0, 2)).astype(ml_dtypes.bfloat16)
    m1t_all = np.ascontiguousarray(mode1.transpose(1, 2, 0))  # [B, D, S]
    m2t_all = np.ascontiguousarray(mode2.transpose(1, 2, 0))

    nc = _get_nc()
    in_maps = []
    for c in range(N_CORES):
        lo, hi = c * BPC, (c + 1) * BPC
        in_maps.append({
            "m1n": m1n_all[lo:hi],
            "m2n": m2n_all[lo:hi],
            "m1t": m1t_all[lo:hi],
            "m2t": m2t_all[lo:hi],
        })

    r = None
    last_err = None
    for attempt in range(3):
        try:
            r = run_bass_kernel_spmd(nc, in_maps, list(range(N_CORES)),
                                     trace=_trace)
            break
        except Exception as e:  # transient NRT exec-unit errors recover on retry
            last_err = e
            time.sleep(2.0)
    if r is None:
        raise last_err
    if _result_box is not None:
        _result_box["result"] = r

    out = np.empty((S, B, 2 * D), dtype=np.float32)
    for c in range(N_CORES):
        res = r.results[c]["out"]  # [BPC, S, 2D]
        out[:, c * BPC:(c + 1) * BPC, :] = res.transpose(1, 0, 2)
    return out


# revision 54
# speedup vs baseline: 1.1712x; 1.0081x over previous
"""BiModalAttention Trainium2 kernel (v4).

Full-input contract: kernel(mode1, mode2) -> [S, B, 2D] float32.
mode1/mode2: [S=1024, B=32, D=1024] float32.

Reference computation per batch b (m1 = mode1[:, b, :], m2 = mode2[:, b, :]):
    C1 = m1 @ m2.T                  # [S, S]
    a1 = softmax_rows(C1) @ m2 * m1
    a2 = softmax_rows(C1.T) @ m1 * m2
    out[:, b, :] = concat([a1, a2], -1)

Sharding: batch dim across 8 NeuronCores (4 batch elements per core).

Design (v4 = measured-best config):
  * Software pipeline keeps the PE dense: phase A(j) interleaves batch j's
    C2-strip transposes/softmax with batch j-1's AV matmuls at instruction
    granularity; phase CP(j) interleaves batch j's fp32r score matmuls
    with batch j-1's E2T build. No keeper matmuls needed in steady state.
  * Per A-section emission order puts the first two AV evacuations ahead
    of the E1T pre-shift adds in the DVE queue, giving the pav PSUM banks
    ~1.8us of slack (was the source of ~1.3us/section PE stalls).
  * Pre-exponent tiles are bf16 (halves DVE write cost; exp(bf16(x)) only
    perturbs softmax weights ~0.2% multiplicatively; verified 3.5e-3
    scale-relative vs the 2e-2 gate).
  * E2T pre-shift adds run on the otherwise-idle Pool engine (SBUF-only:
    GPSIMD cannot access PSUM); the E1T adds read PSUM so they stay on DVE.
  * AV rhs/gate operands (m1n/m2n) are pre-cast to bf16 on the host:
    casting DMAs run ~5GB/s/engine and were saturating the input queue.
  * All input DMA on gpsimd's SWDGE queue (fast descriptor generation),
    emitted exactly where their buffer-reuse waits are already satisfied
    (staggered prefetch); out stores on sync's queue.
"""

import os
os.environ.setdefault("NEURON_RT_RESET_CORES", "1")
import time

import ml_dtypes
import numpy as np

import concourse.bacc as bacc
import concourse.mybir as mybir
import concourse.tile as tile
from concourse.masks import make_identity
from concourse.bass_utils import run_bass_kernel_spmd

S = 1024
D = 1024
B = 32
N_CORES = 8
BPC = B // N_CORES          # batch elements per core
P = 128                     # partitions
NK = S // P                 # contraction tiles (8)
NI = S // P                 # s tiles (8)
CW = 512                    # AV d-chunk width
NCH = D // CW               # AV chunks (2)

f32 = mybir.dt.float32
f32r = mybir.dt.float32r
bf16 = mybir.dt.bfloat16
AX = mybir.AxisListType
ALU = mybir.AluOpType
ACTF = mybir.ActivationFunctionType

E1_BUFS = 12   # e1(j+1)_t reuses e1(j)_(t-4): free after AV(j) dir1 (section 3)
E2_BUFS = 8    # e2(j+1)_i reuses e2(j)_i: AV(j) dir2 done before CP(j+2)
RHS_BUFS = 5   # rotation verified against staggered prefetch points
C1_BUFS = 8


def _emit_m_loads(nc, sb, st, j, m1t, m2t, head):
    """Score operands in d-major [d_part, k, s] layout (f32->f32r casting
    DMA on gpsimd's SWDGE queue — the only engine allowed to cast)."""
    m1t_sb = sb.tile([P, NK, S], f32r, tag="m1t", bufs=1, name=f"m1t{j}")
    m2t_sb = sb.tile([P, NK, S], f32r, tag="m2t", bufs=1, name=f"m2t{j}")
    st["m1t"], st["m2t"] = m1t_sb, m2t_sb
    a1 = m1t[j].rearrange("(k p) s -> p k s", p=P)
    a2 = m2t[j].rearrange("(k p) s -> p k s", p=P)
    if head:
        # batch 0 runs C1 with n outer / i inner; deliver in need-order
        nc.gpsimd.dma_start(out=m1t_sb[:, :, 0:P], in_=a1[:, :, 0:P])
        nc.gpsimd.dma_start(out=m2t_sb[:, :, 0:CW], in_=a2[:, :, 0:CW])
        nc.gpsimd.dma_start(out=m1t_sb[:, :, P:S], in_=a1[:, :, P:S])
        nc.gpsimd.dma_start(out=m2t_sb[:, :, CW:S], in_=a2[:, :, CW:S])
    else:
        nc.gpsimd.dma_start(out=m1t_sb, in_=a1)
        nc.gpsimd.dma_start(out=m2t_sb, in_=a2)


# r-chunk index -> (modality, c): AV rhs/gate tiles, natural [t_part, k, d]
_R_KEYS = (("r2", 0), ("r1", 0), ("r2", 1), ("r1", 1))


def _emit_r_load(nc, sb, st, j, which, m1n, m2n):
    """AV rhs/gate chunk; m1n/m2n are pre-cast to bf16 on the host."""
    key = _R_KEYS[which]
    mn = m1n if key[0] == "r1" else m2n
    c = key[1]
    t = sb.tile([P, NK, CW], bf16, tag="rhs", bufs=RHS_BUFS,
                name=f"r{j}_{which}")
    nc.gpsimd.dma_start(
        out=t,
        in_=mn[j].rearrange("(k p) d -> p k d", p=P)[:, :, c * CW:(c + 1) * CW])
    st[key] = t


def _emit_c1_block(nc, sb, ps, st, j, i, n_list):
    m1t_sb, m2t_sb = st["m1t"], st["m2t"]
    c1 = st.setdefault("c1", {})
    if i not in c1:
        c1[i] = sb.tile([P, S], f32, tag="c1", bufs=C1_BUFS, name=f"c1_{j}_{i}")
    for n in n_list:
        pc = ps.tile([P, CW], f32, tag="pc", bufs=2, name=f"pc{j}_{i}_{n}")
        for k in range(NK):
            nc.tensor.matmul(
                pc,
                m1t_sb[:, k, i * P:(i + 1) * P],
                m2t_sb[:, k, n * CW:(n + 1) * CW],
                start=(k == 0),
                stop=(k == NK - 1),
            )
        nc.scalar.copy(out=c1[i][:, n * CW:(n + 1) * CW], in_=pc)


def _emit_rm1_reduce(nc, sb, st, j, i):
    if "rm1" not in st:
        st["rm1"] = sb.tile([P, NI], f32, tag="rm1", bufs=2, name=f"rm1_{j}")
    nc.vector.tensor_reduce(st["rm1"][:, i:i + 1], st["c1"][i], axis=AX.X,
                            op=ALU.max, negate=True)


def _emit_rmb(nc, sb, ps, ident, st, j, which):
    """Broadcast negated row-max across partitions: rmb[t, s] = rm[s]."""
    rm = st[which]
    rmb = sb.tile([P, S], f32, tag=which + "b", bufs=1, name=f"{which}b_{j}")
    for g in range(2):
        pt = ps.tile([P, CW], f32, tag="pt", bufs=4, name=f"{which}b_pt{j}_{g}")
        for q in range(4):
            i = g * 4 + q
            xb = sb.tile([P, P], f32, tag="xb", bufs=2,
                         name=f"{which}b_xb{j}_{i}")
            nc.vector.tensor_copy(xb, rm[:, i:i + 1].broadcast_to([P, P]))
            nc.tensor.transpose(pt[:, q * P:(q + 1) * P], xb, ident)
        nc.scalar.copy(out=rmb[:, g * CW:(g + 1) * CW], in_=pt)
    st[which + "b"] = rmb


def _emit_trans_part1(nc, sb, ps, ident, st, j, t):
    """C2 strip t: PE transposes -> rm2 partials -> Z2 accum passes."""
    c1 = st["c1"]
    rm2p, rm2, z2p = st["rm2p"], st["rm2"], st["z2p"]
    pts = []
    for g in range(2):
        pt = ps.tile([P, CW], f32, tag="pt", bufs=4, name=f"pt{j}_{t}_{g}")
        pts.append(pt)
        for q in range(4):
            i = g * 4 + q
            nc.tensor.transpose(pt[:, q * P:(q + 1) * P],
                                c1[i][:, t * P:(t + 1) * P], ident)
        nc.vector.tensor_reduce(rm2p[:, 2 * t + g:2 * t + g + 1], pts[g],
                                axis=AX.X, op=ALU.max, negate=True)
    nc.vector.tensor_tensor(rm2[:, t:t + 1], rm2p[:, 2 * t:2 * t + 1],
                            rm2p[:, 2 * t + 1:2 * t + 2], op=ALU.min)
    for g in range(2):
        # Z2 partial straight from PSUM; exp output discarded
        scrz = sb.tile([P, CW], bf16, tag="scr", bufs=2,
                       name=f"scrz_{j}_{t}_{g}")
        nc.scalar.activation(scrz, pts[g], ACTF.Exp, bias=rm2[:, t:t + 1],
                             accum_out=z2p[:, 2 * t + g:2 * t + g + 1])
    return pts


def _emit_trans_part2(nc, sb, st, j, t, pts):
    """E1T strip: pre-shift add (DVE; reads PSUM) + exp. Emitted after two
    AV groups so their evacuations precede these adds in the DVE queue."""
    rm1b = st["rm1b"]
    e1 = st.setdefault("e1", {})
    e1[t] = sb.tile([P, S], bf16, tag="e1", bufs=E1_BUFS, name=f"e1_{j}_{t}")
    epre = sb.tile([P, S], bf16, tag="ep1", bufs=2, name=f"ep1_{j}_{t}")
    for g in range(2):
        nc.vector.tensor_add(epre[:, g * CW:(g + 1) * CW], pts[g],
                             rm1b[:, g * CW:(g + 1) * CW])
    nc.scalar.activation(e1[t], epre, ACTF.Exp)


def _emit_keeper(nc, ps, kc, j, t):
    """Discarded fp32r matmuls keep PE duty high through A(0), where no AV
    work exists to interleave; low-duty epochs there make the HAM run the
    next ~5-9 epochs at half clock INTO the dense CP(1)/A(1) phases
    (observed: 23.9us at 4/8 duty following A(0))."""
    pk = ps.tile([P, 256], f32, tag="pav", bufs=2, name=f"kp{j}_{t}")
    for r in range(6):
        nc.tensor.matmul(pk, kc[:, 0:P], kc, start=(r == 0), stop=(r == 5))


def _emit_scr(nc, sb, st, j, i):
    """Z1[s] accumulation pass over c1 strip i (exp output discarded)."""
    scr = sb.tile([P, S], bf16, tag="scr", bufs=2, name=f"scr_{j}_{i}")
    nc.scalar.activation(scr, st["c1"][i], ACTF.Exp, bias=st["rm1"][:, i:i + 1],
                         accum_out=st["z1"][:, i:i + 1])


def _emit_e2(nc, sb, st, j, i):
    """E2T strip i = exp(C1 - rm2[t]) in [s_part, t] layout (AV stationary).
    The add runs on Pool (SBUF-only operands)."""
    e2 = st.setdefault("e2", {})
    e2[i] = sb.tile([P, S], bf16, tag="e2", bufs=E2_BUFS, name=f"e2_{j}_{i}")
    epre2 = sb.tile([P, S], bf16, tag="ep2", bufs=2, name=f"ep2_{j}_{i}")
    nc.gpsimd.tensor_add(epre2, st["c1"][i], st["rm2b"])
    nc.scalar.activation(e2[i], epre2, ACTF.Exp)


def _emit_av_group(nc, sb, ps, st, j, c, dirx, i, outp):
    if dirx == 1:
        es, rhs, gate, invz, dbase = (st["e1"], st[("r2", c)], st[("r1", c)],
                                      st["invz1"], 0)
    else:
        es, rhs, gate, invz, dbase = (st["e2"], st[("r1", c)], st[("r2", c)],
                                      st["invz2"], D)
    pav = ps.tile([P, CW], f32, tag="pav", bufs=2, name=f"pav{j}_{c}_{dirx}_{i}")
    for k in range(NK):
        nc.tensor.matmul(
            pav,
            es[k][:, i * P:(i + 1) * P],
            rhs[:, k, :],
            start=(k == 0),
            stop=(k == NK - 1),
        )
    a_sb = sb.tile([P, CW], f32, tag="ao", bufs=4, name=f"a{j}_{c}_{dirx}_{i}")
    nc.vector.scalar_tensor_tensor(a_sb, pav, invz[:, i:i + 1], gate[:, i, :],
                                   op0=ALU.mult, op1=ALU.mult)
    nc.sync.dma_start(
        out=outp[j, i * P:(i + 1) * P, dbase + c * CW:dbase + (c + 1) * CW],
        in_=a_sb)


def _av_group_list():
    return [(c, dirx, i)
            for (c, dirx) in ((0, 1), (1, 1), (0, 2), (1, 2))
            for i in range(NI)]


def _emit_A(nc, sb, ps, ident, sts, j, outp, m1n, m2n, m1t, m2t):
    """Phase A(j): transposes/softmax of batch j + AV of batch j-1,
    interleaved 1 transpose-group : 4 AV-groups per section."""
    st = sts[j]
    st["rm2p"] = sb.tile([P, 2 * NK], f32, tag="rm2p", bufs=2, name=f"rm2p_{j}")
    st["rm2"] = sb.tile([P, NK], f32, tag="rm2", bufs=2, name=f"rm2_{j}")
    st["z2p"] = sb.tile([P, 2 * NK], f32, tag="z2p", bufs=2, name=f"z2p_{j}")
    st["z2"] = sb.tile([P, NK], f32, tag="z2", bufs=2, name=f"z2_{j}")
    st["z1"] = sb.tile([P, NI], f32, tag="z1", bufs=2, name=f"z1_{j}")
    _emit_rmb(nc, sb, ps, ident, st, j, "rm1")
    if j + 1 < BPC:
        # m(j+1) loads: WAR on C1(j) matmuls releases right about now, so
        # this never head-of-line-blocks Pool compute behind it
        _emit_m_loads(nc, sb, sts[j + 1], j + 1, m1t, m2t, head=False)

    av = _av_group_list() if j >= 1 else []
    stp = sts[j - 1] if j >= 1 else None
    gi = 0
    for t in range(NK):
        pts = _emit_trans_part1(nc, sb, ps, ident, st, j, t)
        for _ in range(2):
            if gi < len(av):
                c, dirx, i = av[gi]
                _emit_av_group(nc, sb, ps, stp, j - 1, c, dirx, i, outp)
                gi += 1
        if not av:
            _emit_keeper(nc, ps, st["kc"], j, t)
        _emit_trans_part2(nc, sb, st, j, t, pts)
        _emit_scr(nc, sb, st, j, t)
        for _ in range(2):
            if gi < len(av):
                c, dirx, i = av[gi]
                _emit_av_group(nc, sb, ps, stp, j - 1, c, dirx, i, outp)
                gi += 1
        if t == 5 and j >= 1:
            # r(j)_1 / r(j)_2 slots freed by AV(j-1) c0-dir2 (section 5)
            _emit_r_load(nc, sb, st, j, 1, m1n, m2n)
            _emit_r_load(nc, sb, st, j, 2, m1n, m2n)
    _emit_rmb(nc, sb, ps, ident, st, j, "rm2")
    for t in range(NK):
        nc.vector.tensor_tensor(st["z2"][:, t:t + 1],
                                st["z2p"][:, 2 * t:2 * t + 1],
                                st["z2p"][:, 2 * t + 1:2 * t + 2], op=ALU.add)
    st["invz1"] = sb.tile([P, NI], f32, tag="invz1", bufs=2, name=f"invz1_{j}")
    st["invz2"] = sb.tile([P, NI], f32, tag="invz2", bufs=2, name=f"invz2_{j}")
    nc.vector.reciprocal(st["invz2"], st["z2"])
    nc.vector.reciprocal(st["invz1"], st["z1"])
    if j >= 1:
        # r(j)_3 slot freed by AV(j-1) c1-dir2 (section 7)
        _emit_r_load(nc, sb, st, j, 3, m1n, m2n)


def _emit_CP(nc, sb, ps, sts, j, m1t, m2t, m1n, m2n):
    """Phase CP(j): C1 scores of batch j + E2T build of batch j-1."""
    st, stp = sts[j], sts[j - 1]
    _emit_r_load(nc, sb, st, j, 0, m1n, m2n)
    for i in range(NI):
        _emit_e2(nc, sb, stp, j - 1, i)
        _emit_c1_block(nc, sb, ps, st, j, i, (0, 1))
        _emit_rm1_reduce(nc, sb, st, j, i)


def _emit_tail(nc, sb, ps, sts, outp):
    """E2T(last) build interleaved with AV(last) dir1; then dir2."""
    j = BPC - 1
    st = sts[j]
    dir1 = [(c, 1, i) for c in (0, 1) for i in range(NI)]
    gi = 0
    for i in range(NI):
        _emit_e2(nc, sb, st, j, i)
        for _ in range(2):
            c, dirx, ii = dir1[gi]
            _emit_av_group(nc, sb, ps, st, j, c, dirx, ii, outp)
            gi += 1
    for c in (0, 1):
        for i in range(NI):
            _emit_av_group(nc, sb, ps, st, j, c, 2, i, outp)


def _build():
    nc = bacc.Bacc("TRN2", target_bir_lowering=False, debug=False,
                   num_devices=N_CORES)
    m1n = nc.dram_tensor("m1n", [BPC, S, D], bf16, kind="ExternalInput").ap()
    m2n = nc.dram_tensor("m2n", [BPC, S, D], bf16, kind="ExternalInput").ap()
    m1t = nc.dram_tensor("m1t", [BPC, D, S], f32, kind="ExternalInput").ap()
    m2t = nc.dram_tensor("m2t", [BPC, D, S], f32, kind="ExternalInput").ap()
    outp = nc.dram_tensor("out", [BPC, S, 2 * D], f32, kind="ExternalOutput").ap()

    with tile.TileContext(nc) as tc:
        with tc.tile_pool(name="consts", bufs=1) as consts, \
             tc.tile_pool(name="sb", bufs=1) as sb, \
             tc.tile_pool(name="ps", bufs=1, space="PSUM") as ps:
            sts = [dict() for _ in range(BPC)]

            # head loads first so transfers start ASAP
            _emit_m_loads(nc, sb, sts[0], 0, m1t, m2t, head=True)
            for w in range(4):
                _emit_r_load(nc, sb, sts[0], 0, w, m1n, m2n)

            ident = consts.tile([P, P], f32)
            make_identity(nc, ident)
            kc = consts.tile([P, 256], f32r)
            nc.vector.memset(kc.bitcast(f32), 1.0)
            sts[0]["kc"] = kc

            # CP(0): C1(0), n outer so compute starts after ~2.5MB of DMA
            for n in (0, 1):
                for i in range(NI):
                    _emit_c1_block(nc, sb, ps, sts[0], 0, i, (n,))
                    if n == 1:
                        _emit_rm1_reduce(nc, sb, sts[0], 0, i)

            _emit_A(nc, sb, ps, ident, sts, 0, outp, m1n, m2n, m1t, m2t)
            for j in range(1, BPC):
                _emit_CP(nc, sb, ps, sts, j, m1t, m2t, m1n, m2n)
                _emit_A(nc, sb, ps, ident, sts, j, outp, m1n, m2n, m1t, m2t)
            _emit_tail(nc, sb, ps, sts, outp)
    nc.compile()
    return nc


_NC_CACHE = None


def _get_nc():
    global _NC_CACHE
    if _NC_CACHE is None:
        _NC_CACHE = _build()
    return _NC_CACHE


def kernel(mode1: np.ndarray, mode2: np.ndarray, _trace: bool = False,
           _result_box: dict | None = None) -> np.ndarray:
    mode1 = np.asarray(mode1, dtype=np.float32)
    mode2 = np.asarray(mode2, dtype=np.float32)

    m1n_all = np.ascontiguousarray(
        mode1.transpose(1, 0, 2)).astype(ml_dtypes.bfloat16)  # [B, S, D]
    m2n_all = np.ascontiguousarray(
        mode2.transpose(1, 0, 2)).astype(ml_dtypes.bfloat16)
    m1t_all = np.ascontiguousarray(mode1.transpose(1, 2, 0))  # [B, D, S]
    m2t_all = np.ascontiguousarray(mode2.transpose(1, 2, 0))

    nc = _get_nc()
    in_maps = []
    for c in range(N_CORES):
        lo, hi = c * BPC, (c + 1) * BPC
        in_maps.append({
            "m1n": m1n_all[lo:hi],
            "m2n": m2n_all[lo:hi],
            "m1t": m1t_all[lo:hi],
            "m2t": m2t_all[lo:hi],
        })

    r = None
    last_err = None
    for attempt in range(3):
        try:
            r = run_bass_kernel_spmd(nc, in_maps, list(range(N_CORES)),
                                     trace=_trace)
            break
        except Exception as e:  # transient NRT exec-unit errors recover on retry
            last_err = e
            time.sleep(2.0)
    if r is None:
        raise last_err
    if _result_box is not None:
        _result_box["result"] = r

    out = np.empty((S, B, 2 * D), dtype=np.float32)
    for c in range(N_CORES):
        res = r.results[c]["out"]  # [BPC, S, 2D]
        out[:, c * BPC:(c + 1) * BPC, :] = res.transpose(1, 0, 2)
    return out
